# revision 23
# baseline (speedup 1.0000x reference)
"""HGConv fused kernel for one TRN2 chip (8 NeuronCores), SPMD via Bass/Tile.

Hardcoded for M=16384 nodes, E=4096 hyperedges, D=300, N_CAT=3, 8 cores.

Edge-sharded design (no mid-kernel ReduceScatter):
  - Core c owns edges [512c, 512(c+1)).  It streams the FULL node_feats X
    (fp16, replicated) plus its inc column-slice (fp16, host-pre-tiled so
    every DMA descriptor is a multi-KB contiguous run) and accumulates
    IX[e_c, :] = inc[:, e_c].T @ X locally in 4 PSUM banks (4 e-subtiles
    x 128 edges, contraction over all 128 m-tiles).
  - Tail on the 512 local edges: att = IX @ W_att (reassociated), softmax
    over d (stabilized), ef = IX * attn; ef2T = (1-a)*W_proj.T @ efT +
    a*edge_feats.T (edge_feats pre-transposed/pre-scaled on host);
    scores|G = ef2T.T @ [ec_W_att | ec_W_proj @ fc_W] (classifier weights
    folded on host); unstabilized exp(score) (scores are O(5), fp32-safe);
    p2|z accumulated with a PE matmul against [G | ones].
  - AllGather of the per-core 8-float partials; every core redundantly
    combines (sum / z) and adds the folded bias to produce the (3,) logits.
"""

import sys

for _p in ("/opt/trn_rl_repo", "/opt/pypackages"):
    if _p not in sys.path:
        sys.path.append(_p)

import numpy as np

import concourse.bacc as bacc
import concourse.tile as tile
from concourse import masks, mybir
from concourse.bass_utils import run_bass_kernel_spmd

F32 = mybir.dt.float32
F32R = mybir.dt.float32r
F16 = mybir.dt.float16
AX = mybir.AxisListType
OP = mybir.AluOpType
AF = mybir.ActivationFunctionType

NCORES = 8
M, E, D, NCAT = 16384, 4096, 300, 3
E_SH = E // NCORES          # 512 edges per core
ET_SH = E_SH // 128         # 4 e-subtiles per core
MT_TOT = M // 128           # 128 m-tiles over the full node axis
MCH = 8                     # m-tiles per streamed chunk
NCH = MT_TOT // MCH         # 16 chunks
DCH = (128, 128, 44)        # d split into partition chunks
DOF = (0, 128, 256)


def _build():
    nc = bacc.Bacc("TRN2", target_bir_lowering=False, debug=False,
                   num_devices=NCORES)
    x_d = nc.dram_tensor("x", [128, MT_TOT, D], F16, kind="ExternalInput")
    inc_d = nc.dram_tensor("inc", [128, MT_TOT, E_SH], F16,
                           kind="ExternalInput")
    efT_d = nc.dram_tensor("efT", [128, 3, E_SH], F16, kind="ExternalInput")
    watt_d = nc.dram_tensor("watt", [D, D], F16, kind="ExternalInput")
    wproj_d = nc.dram_tensor("wproj", [D, D], F16, kind="ExternalInput")
    sgw_d = nc.dram_tensor("sgw", [D, 4], F16, kind="ExternalInput")
    out_d = nc.dram_tensor("out", [1, 8], F32, kind="ExternalOutput")

    def mm(out, lhsT, rhs, start, stop):
        nc.tensor.matmul(out, lhsT, rhs, start=start, stop=stop)

    def r(ap):  # reinterpret f32 data as f32r for full-rate matmul
        return ap.bitcast(F32R)

    with tile.TileContext(nc) as tc, \
         tc.tile_pool(name="sb", bufs=1) as sb, \
         tc.tile_pool(name="dram", bufs=1, space="DRAM") as dram:

        # ---------- phase 1: IX[e_c, :] = inc_c.T @ X (full m) ----------
        with tc.tile_pool(name="pacc", bufs=ET_SH, space="PSUM") as pacc, \
             tc.tile_pool(name="xp", bufs=8) as xp, \
             tc.tile_pool(name="ip", bufs=8) as ip:
            acc = [pacc.tile([128, D], F32, name=f"acc{es}", tag="acc")
                   for es in range(ET_SH)]
            # small leading chunks so the PE starts as soon as possible
            sizes = [2, 2, 4] + [MCH] * ((MT_TOT - 8) // MCH)
            assert sum(sizes) == MT_TOT
            mt0 = 0
            for ch, csz in enumerate(sizes):
                i_t = ip.tile([128, csz, E_SH], F16, tag="inc",
                              name=f"inc_c{ch}")
                x_t = xp.tile([128, csz, D], F16, tag="x", name=f"x_c{ch}")
                nc.sync.dma_start(i_t[:], inc_d[:, mt0:mt0 + csz, :])
                nc.scalar.dma_start(x_t[:], x_d[:, mt0:mt0 + csz, :])
                for mt in range(csz):
                    for es in range(ET_SH):
                        mm(acc[es][:],
                           i_t[:, mt, es * 128:(es + 1) * 128],
                           x_t[:, mt, :],
                           start=(mt0 + mt == 0),
                           stop=(mt0 + mt == MT_TOT - 1))
                mt0 += csz

            # ---------- small weights / constants (overlap phase 1) ------
            watt_sb = sb.tile([128, 3, D], F16)
            wproj_sb = sb.tile([128, 3, D], F16)
            sgw_sb = sb.tile([128, 3, 4], F16)
            for i, (c, o) in enumerate(zip(DCH, DOF)):
                nc.gpsimd.dma_start(watt_sb[:c, i, :], watt_d[o:o + c, :])
                nc.gpsimd.dma_start(wproj_sb[:c, i, :], wproj_d[o:o + c, :])
                nc.gpsimd.dma_start(sgw_sb[:c, i, :], sgw_d[o:o + c, :])
            efT_sb = sb.tile([128, 3, E_SH], F16)
            nc.gpsimd.dma_start(efT_sb[:], efT_d[:])

            # IX psum -> sbuf (inside pacc scope, then release its banks)
            def cp(k, dst, src):
                # psum-reading copies: only ACT/DVE may touch PSUM
                e = (nc.scalar.copy, nc.vector.tensor_copy)[k % 2]
                e(dst, src)

            DP = 384                        # d padded to a 128 multiple
            ix_sb = sb.tile([128, ET_SH, DP], F16)
            nc.vector.memset(ix_sb[:, :, D:DP], 0.0)
            for es in range(ET_SH):
                cp(es, ix_sb[:, es, 0:D], acc[es][:])

        # ---------- tail on this core's 512 edges ----------
        if True:
            with tc.tile_pool(name="pp", bufs=4, space="PSUM") as pp:
                def transpose_512xD(src_sb, dstT_sb):
                    # src (128, 4, 384) f16 [e-part] -> dstT (128, 3, 512)
                    # f16 [d-part] via the DMA xbar (16x128 tiles); the
                    # i=2 chunk carries junk rows 44:128, never read.
                    k = 0
                    for et in range(ET_SH):
                        for i in range(3):
                            eng = (nc.sync, nc.scalar)[k % 2]
                            eng.dma_start_transpose(
                                dstT_sb[:, i, et * 128:(et + 1) * 128],
                                src_sb[:, et, i * 128:(i + 1) * 128])
                            k += 1

                ixT_sb = sb.tile([128, 3, E_SH], F16)
                transpose_512xD(ix_sb, ixT_sb)

                # edge_att = IX @ W_att; softmax over d; ef = IX * attn
                ef_sb = sb.tile([128, ET_SH, DP], F16)
                nc.vector.memset(ef_sb[:, :, D:DP], 0.0)
                stat_sb = sb.tile([128, ET_SH, 4], F32)
                for et in range(ET_SH):
                    att = pp.tile([128, D], F32, tag="ps")
                    for i, c in enumerate(DCH):
                        mm(att[:], ixT_sb[:c, i, et * 128:(et + 1) * 128],
                           watt_sb[:c, i, :], start=(i == 0), stop=(i == 2))
                    nmax = stat_sb[:, et, 0:1]
                    nc.vector.tensor_reduce(nmax, att[:], axis=AX.X,
                                            op=OP.max, negate=True)
                    ex = pp.tile([128, D], F32, tag="ps")
                    rsum = stat_sb[:, et, 1:2]
                    nc.scalar.activation(ex[:], att[:], AF.Exp, bias=nmax,
                                         scale=1.0, accum_out=rsum)
                    rcp = stat_sb[:, et, 2:3]
                    nc.vector.reciprocal(rcp, rsum)
                    nc.vector.scalar_tensor_tensor(
                        ef_sb[:, et, 0:D], ex[:], rcp, ix_sb[:, et, 0:D],
                        op0=OP.mult, op1=OP.mult)

                efTT_sb = sb.tile([128, 3, E_SH], F16)
                transpose_512xD(ef_sb, efTT_sb)

                # ef2T = (1-a)*W_proj.T @ efT + a*edge_feats.T
                ef2T_sb = sb.tile([128, 3, E_SH], F16)
                for i, (c, o) in enumerate(zip(DCH, DOF)):
                    pj = pp.tile([128, E_SH], F32, tag="ps")
                    for j, cj in enumerate(DCH):
                        mm(pj[:c, :], wproj_sb[:cj, j, o:o + c],
                           efTT_sb[:cj, j, :], start=(j == 0),
                           stop=(j == 2))
                    nc.vector.tensor_add(ef2T_sb[:c, i, :], pj[:c, :],
                                         efT_sb[:c, i, :])

                # scores|G = ef2 @ [ec_W_att | ec_W_proj @ fc_W]  -> (e, 4)
                g_sb = sb.tile([128, ET_SH, 8], F32)
                nc.vector.memset(g_sb[:, :, 4:5], 1.0)
                expw_sb = sb.tile([128, ET_SH], F32)
                for et in range(ET_SH):
                    sg = pp.tile([128, 4], F32, tag="ps")
                    for j, cj in enumerate(DCH):
                        mm(sg[:], ef2T_sb[:cj, j, et * 128:(et + 1) * 128],
                           sgw_sb[:cj, j, :], start=(j == 0), stop=(j == 2))
                    nc.scalar.copy(g_sb[:, et, 0:4], sg[:])
                    nc.scalar.activation(expw_sb[:, et:et + 1],
                                         g_sb[:, et, 0:1], AF.Exp, scale=1.0)

                # p2|z = sum_e exp_e * [G_e | 1]   (PE contraction over e)
                p2 = pp.tile([1, 4], F32, tag="p2")
                for et in range(ET_SH):
                    mm(p2[:], expw_sb[:, et:et + 1], g_sb[:, et, 1:5],
                       start=(et == 0), stop=(et == ET_SH - 1))
                # per-core partial [p2(3) | z | pad]; host sums over cores
                # and finishes logits = sum(p2)/sum(z) + b2 (gather/unshard)
                pk_sb = sb.tile([1, 8], F32)
                nc.vector.memset(pk_sb[:, 4:8], 0.0)
                nc.scalar.copy(pk_sb[:, 0:4], p2[:])
                nc.sync.dma_start(out_d[:], pk_sb[0:1, :])

    nc.compile()
    return nc


_CACHE = {}


def get_nc():
    if "nc" not in _CACHE:
        _CACHE["nc"] = _build()
    return _CACHE["nc"]


def make_in_maps(node_feats, edge_feats, inc_mat, W_att, W_proj, alpha,
                 ec_W_att, ec_W_proj, ec_b_proj, fc_W, fc_b):
    cc = lambda a: np.ascontiguousarray(np.asarray(a, np.float32))
    node_feats = cc(node_feats)
    inc_mat = cc(inc_mat)
    edge_feats = cc(edge_feats)
    W_att, W_proj = cc(W_att), cc(W_proj)
    ec_W_att, ec_W_proj = cc(ec_W_att).reshape(D, 1), cc(ec_W_proj)
    ec_b_proj, fc_W, fc_b = cc(ec_b_proj), cc(fc_W), cc(fc_b)
    a = float(np.asarray(alpha))

    # x packed [p, mt, d] fp16, replicated
    x_pack = np.ascontiguousarray(
        node_feats.reshape(MT_TOT, 128, D).transpose(1, 0, 2)
    ).astype(np.float16)
    # folded weights
    G2 = ec_W_proj @ fc_W                     # (300, 3)
    sgw = np.ascontiguousarray(
        np.concatenate([ec_W_att, G2], axis=1))  # (300, 4)
    b2 = ec_b_proj @ fc_W + fc_b              # (3,)
    wproj_s = np.ascontiguousarray((1.0 - a) * W_proj).astype(np.float16)
    common = dict(x=x_pack, watt=W_att.astype(np.float16), wproj=wproj_s,
                  sgw=sgw.astype(np.float16))

    in_maps = []
    for c in range(NCORES):
        sl = slice(c * E_SH, (c + 1) * E_SH)
        # rotate the m-tile order per core so the 8 cores never stream the
        # same region of the replicated x at the same instant (HBM hotspot)
        rot = np.roll(np.arange(MT_TOT), -c * (MT_TOT // NCORES))
        inc_pack = np.ascontiguousarray(
            inc_mat[:, sl].reshape(MT_TOT, 128, E_SH)[rot].transpose(1, 0, 2)
        ).astype(np.float16)
        x_rot = np.ascontiguousarray(common["x"][:, rot, :])
        efT = np.zeros((128, 3, E_SH), np.float16)
        eft_full = a * edge_feats[sl].T       # (300, 512), pre-scaled
        for i, (cch, o) in enumerate(zip(DCH, DOF)):
            efT[:cch, i, :] = eft_full[o:o + cch, :]
        in_maps.append(dict(inc=inc_pack, efT=efT,
                            **{k: v for k, v in common.items() if k != "x"},
                            x=x_rot))
    return in_maps


def kernel(node_feats, edge_feats, inc_mat, W_att, W_proj, alpha,
           ec_W_att, ec_W_proj, ec_b_proj, fc_W, fc_b, trace=False):
    nc = get_nc()
    in_maps = make_in_maps(node_feats, edge_feats, inc_mat, W_att, W_proj,
                           alpha, ec_W_att, ec_W_proj, ec_b_proj, fc_W, fc_b)
    res = run_bass_kernel_spmd(nc, in_maps, list(range(NCORES)), trace=trace)
    kernel.last_results = res
    parts = np.stack([np.asarray(r["out"], np.float64).reshape(8)
                      for r in res.results]).sum(axis=0)
    a = float(np.asarray(alpha))
    ec_W_proj = np.asarray(ec_W_proj, np.float64)
    b2 = np.asarray(ec_b_proj, np.float64) @ np.asarray(fc_W, np.float64) \
        + np.asarray(fc_b, np.float64)
    b2 = b2.reshape(NCAT)
    logits = parts[0:NCAT] / parts[NCAT] + b2
    return logits.astype(np.float32)


# revision 25
# speedup vs baseline: 1.0485x; 1.0485x over previous
"""HGConv fused kernel for one TRN2 chip (8 NeuronCores), SPMD via Bass/Tile.

Hardcoded for M=16384 nodes, E=4096 hyperedges, D=300, N_CAT=3, 8 cores.

Edge-sharded design (no mid-kernel ReduceScatter):
  - Core c owns edges [512c, 512(c+1)).  It streams the FULL node_feats X
    (fp16, replicated) plus its inc column-slice (fp16, host-pre-tiled so
    every DMA descriptor is a multi-KB contiguous run) and accumulates
    IX[e_c, :] = inc[:, e_c].T @ X locally in 4 PSUM banks (4 e-subtiles
    x 128 edges, contraction over all 128 m-tiles).
  - Tail on the 512 local edges: att = IX @ W_att (reassociated), softmax
    over d (stabilized), ef = IX * attn; ef2T = (1-a)*W_proj.T @ efT +
    a*edge_feats.T (edge_feats pre-transposed/pre-scaled on host);
    scores|G = ef2T.T @ [ec_W_att | ec_W_proj @ fc_W] (classifier weights
    folded on host); unstabilized exp(score) (scores are O(5), fp32-safe);
    p2|z accumulated with a PE matmul against [G | ones].
  - AllGather of the per-core 8-float partials; every core redundantly
    combines (sum / z) and adds the folded bias to produce the (3,) logits.
"""

import sys

for _p in ("/opt/trn_rl_repo", "/opt/pypackages"):
    if _p not in sys.path:
        sys.path.append(_p)

import numpy as np

import concourse.bacc as bacc
import concourse.tile as tile
from concourse import masks, mybir
from concourse.bass_utils import run_bass_kernel_spmd

F32 = mybir.dt.float32
F32R = mybir.dt.float32r
F16 = mybir.dt.float16
AX = mybir.AxisListType
OP = mybir.AluOpType
AF = mybir.ActivationFunctionType

NCORES = 8
M, E, D, NCAT = 16384, 4096, 300, 3
E_SH = E // NCORES          # 512 edges per core
ET_SH = E_SH // 128         # 4 e-subtiles per core
MT_TOT = M // 128           # 128 m-tiles over the full node axis
MCH = 8                     # m-tiles per streamed chunk
NCH = MT_TOT // MCH         # 16 chunks
DCH = (128, 128, 44)        # d split into partition chunks
DOF = (0, 128, 256)


def _build():
    nc = bacc.Bacc("TRN2", target_bir_lowering=False, debug=False,
                   num_devices=NCORES)
    x_d = nc.dram_tensor("x", [128, MT_TOT, D], F16, kind="ExternalInput")
    inc_d = nc.dram_tensor("inc", [128, MT_TOT, E_SH], F16,
                           kind="ExternalInput")
    efT_d = nc.dram_tensor("efT", [128, 3, E_SH], F16, kind="ExternalInput")
    watt_d = nc.dram_tensor("watt", [D, D], F16, kind="ExternalInput")
    wproj_d = nc.dram_tensor("wproj", [D, D], F16, kind="ExternalInput")
    sgw_d = nc.dram_tensor("sgw", [D, 4], F16, kind="ExternalInput")
    out_d = nc.dram_tensor("out", [1, 8], F32, kind="ExternalOutput")

    def mm(out, lhsT, rhs, start, stop):
        nc.tensor.matmul(out, lhsT, rhs, start=start, stop=stop)

    def r(ap):  # reinterpret f32 data as f32r for full-rate matmul
        return ap.bitcast(F32R)

    with tile.TileContext(nc) as tc, \
         tc.tile_pool(name="sb", bufs=1) as sb, \
         tc.tile_pool(name="dram", bufs=1, space="DRAM") as dram:

        # ---------- phase 1: IX[e_c, :] = inc_c.T @ X (full m) ----------
        with tc.tile_pool(name="pacc", bufs=ET_SH, space="PSUM") as pacc, \
             tc.tile_pool(name="xp", bufs=8) as xp, \
             tc.tile_pool(name="ip", bufs=8) as ip:
            acc = [pacc.tile([128, D], F32, name=f"acc{es}", tag="acc")
                   for es in range(ET_SH)]
            # small leading chunks so the PE starts as soon as possible
            sizes = [2, 2, 4] + [MCH] * ((MT_TOT - 8) // MCH)
            assert sum(sizes) == MT_TOT
            mt0 = 0
            for ch, csz in enumerate(sizes):
                i_t = ip.tile([128, csz, E_SH], F16, tag="inc",
                              name=f"inc_c{ch}")
                x_t = xp.tile([128, csz, D], F16, tag="x", name=f"x_c{ch}")
                nc.sync.dma_start(i_t[:], inc_d[:, mt0:mt0 + csz, :])
                nc.scalar.dma_start(x_t[:], x_d[:, mt0:mt0 + csz, :])
                for mt in range(csz):
                    for es in range(ET_SH):
                        mm(acc[es][:],
                           i_t[:, mt, es * 128:(es + 1) * 128],
                           x_t[:, mt, :],
                           start=(mt0 + mt == 0),
                           stop=(mt0 + mt == MT_TOT - 1))
                mt0 += csz

            # ---------- small weights / constants (overlap phase 1) ------
            watt_sb = sb.tile([128, 3, D], F16)
            wproj_sb = sb.tile([128, 3, D], F16)
            sgw_sb = sb.tile([128, 3, 4], F16)
            for i, (c, o) in enumerate(zip(DCH, DOF)):
                nc.gpsimd.dma_start(watt_sb[:c, i, :], watt_d[o:o + c, :])
                nc.gpsimd.dma_start(wproj_sb[:c, i, :], wproj_d[o:o + c, :])
                nc.gpsimd.dma_start(sgw_sb[:c, i, :], sgw_d[o:o + c, :])
            efT_sb = sb.tile([128, 3, E_SH], F16)
            nc.gpsimd.dma_start(efT_sb[:], efT_d[:])
            ident = sb.tile([128, 128], F16)
            masks.make_identity(nc, ident[:])

            # IX psum -> sbuf (inside pacc scope, then release its banks)
            def cp(k, dst, src):
                # psum-reading copies: only ACT/DVE may touch PSUM
                e = (nc.scalar.copy, nc.vector.tensor_copy)[k % 2]
                e(dst, src)

            DP = 384                        # d padded to a 128 multiple
            ix_sb = sb.tile([128, ET_SH, DP], F16)
            nc.vector.memset(ix_sb[:, :, D:DP], 0.0)
            for es in range(ET_SH):
                cp(es, ix_sb[:, es, 0:D], acc[es][:])

        # ---------- tail on this core's 512 edges ----------
        if True:
            with tc.tile_pool(name="pp", bufs=4, space="PSUM") as pp:
                def transpose_512xD(src_sb, dstT_sb):
                    # src (128, 4, 384) f16 [e-part] -> dstT (128, 3, 512)
                    # f16 [d-part]; PE transpose (f16, 1cyc/row) + cast copy
                    k = 0
                    for et in range(ET_SH):
                        for i, (c, o) in enumerate(zip(DCH, DOF)):
                            tp = pp.tile([128, 128], F16, tag="ps")
                            nc.tensor.transpose(tp[:c, :128],
                                                src_sb[:, et, o:o + c],
                                                ident[:])
                            cp(k, dstT_sb[:c, i, et * 128:(et + 1) * 128],
                               tp[:c, :128])
                            k += 1

                ixT_sb = sb.tile([128, 3, E_SH], F16)
                transpose_512xD(ix_sb, ixT_sb)

                # edge_att = IX @ W_att; softmax over d; ef = IX * attn
                ef_sb = sb.tile([128, ET_SH, DP], F16)
                nc.vector.memset(ef_sb[:, :, D:DP], 0.0)
                stat_sb = sb.tile([128, ET_SH, 4], F32)
                for et in range(ET_SH):
                    att = pp.tile([128, D], F32, tag="ps")
                    for i, c in enumerate(DCH):
                        mm(att[:], ixT_sb[:c, i, et * 128:(et + 1) * 128],
                           watt_sb[:c, i, :], start=(i == 0), stop=(i == 2))
                    nmax = stat_sb[:, et, 0:1]
                    nc.vector.tensor_reduce(nmax, att[:], axis=AX.X,
                                            op=OP.max, negate=True)
                    ex = pp.tile([128, D], F32, tag="ps")
                    rsum = stat_sb[:, et, 1:2]
                    nc.scalar.activation(ex[:], att[:], AF.Exp, bias=nmax,
                                         scale=1.0, accum_out=rsum)
                    rcp = stat_sb[:, et, 2:3]
                    nc.vector.reciprocal(rcp, rsum)
                    nc.vector.scalar_tensor_tensor(
                        ef_sb[:, et, 0:D], ex[:], rcp, ix_sb[:, et, 0:D],
                        op0=OP.mult, op1=OP.mult)

                efTT_sb = sb.tile([128, 3, E_SH], F16)
                transpose_512xD(ef_sb, efTT_sb)

                # ef2T = (1-a)*W_proj.T @ efT + a*edge_feats.T
                ef2T_sb = sb.tile([128, 3, E_SH], F16)
                for i, (c, o) in enumerate(zip(DCH, DOF)):
                    pj = pp.tile([128, E_SH], F32, tag="ps")
                    for j, cj in enumerate(DCH):
                        mm(pj[:c, :], wproj_sb[:cj, j, o:o + c],
                           efTT_sb[:cj, j, :], start=(j == 0),
                           stop=(j == 2))
                    nc.vector.tensor_add(ef2T_sb[:c, i, :], pj[:c, :],
                                         efT_sb[:c, i, :])

                # scores|G = ef2 @ [ec_W_att | ec_W_proj @ fc_W]  -> (e, 4)
                g_sb = sb.tile([128, ET_SH, 8], F32)
                nc.vector.memset(g_sb[:, :, 4:5], 1.0)
                expw_sb = sb.tile([128, ET_SH], F32)
                for et in range(ET_SH):
                    sg = pp.tile([128, 4], F32, tag="ps")
                    for j, cj in enumerate(DCH):
                        mm(sg[:], ef2T_sb[:cj, j, et * 128:(et + 1) * 128],
                           sgw_sb[:cj, j, :], start=(j == 0), stop=(j == 2))
                    nc.scalar.copy(g_sb[:, et, 0:4], sg[:])
                    nc.scalar.activation(expw_sb[:, et:et + 1],
                                         g_sb[:, et, 0:1], AF.Exp, scale=1.0)

                # p2|z = sum_e exp_e * [G_e | 1]   (PE contraction over e)
                p2 = pp.tile([1, 4], F32, tag="p2")
                for et in range(ET_SH):
                    mm(p2[:], expw_sb[:, et:et + 1], g_sb[:, et, 1:5],
                       start=(et == 0), stop=(et == ET_SH - 1))
                # per-core partial [p2(3) | z | pad]; host sums over cores
                # and finishes logits = sum(p2)/sum(z) + b2 (gather/unshard)
                pk_sb = sb.tile([1, 8], F32)
                nc.vector.memset(pk_sb[:, 4:8], 0.0)
                nc.scalar.copy(pk_sb[:, 0:4], p2[:])
                nc.sync.dma_start(out_d[:], pk_sb[0:1, :])

    nc.compile()
    return nc


_CACHE = {}


def get_nc():
    if "nc" not in _CACHE:
        _CACHE["nc"] = _build()
    return _CACHE["nc"]


def make_in_maps(node_feats, edge_feats, inc_mat, W_att, W_proj, alpha,
                 ec_W_att, ec_W_proj, ec_b_proj, fc_W, fc_b):
    cc = lambda a: np.ascontiguousarray(np.asarray(a, np.float32))
    node_feats = cc(node_feats)
    inc_mat = cc(inc_mat)
    edge_feats = cc(edge_feats)
    W_att, W_proj = cc(W_att), cc(W_proj)
    ec_W_att, ec_W_proj = cc(ec_W_att).reshape(D, 1), cc(ec_W_proj)
    ec_b_proj, fc_W, fc_b = cc(ec_b_proj), cc(fc_W), cc(fc_b)
    a = float(np.asarray(alpha))

    # x packed [p, mt, d] fp16, replicated
    x_pack = np.ascontiguousarray(
        node_feats.reshape(MT_TOT, 128, D).transpose(1, 0, 2)
    ).astype(np.float16)
    # folded weights
    G2 = ec_W_proj @ fc_W                     # (300, 3)
    sgw = np.ascontiguousarray(
        np.concatenate([ec_W_att, G2], axis=1))  # (300, 4)
    b2 = ec_b_proj @ fc_W + fc_b              # (3,)
    wproj_s = np.ascontiguousarray((1.0 - a) * W_proj).astype(np.float16)
    common = dict(x=x_pack, watt=W_att.astype(np.float16), wproj=wproj_s,
                  sgw=sgw.astype(np.float16))

    in_maps = []
    for c in range(NCORES):
        sl = slice(c * E_SH, (c + 1) * E_SH)
        # rotate the m-tile order per core so the 8 cores never stream the
        # same region of the replicated x at the same instant (HBM hotspot)
        rot = np.roll(np.arange(MT_TOT), -c * (MT_TOT // NCORES))
        inc_pack = np.ascontiguousarray(
            inc_mat[:, sl].reshape(MT_TOT, 128, E_SH)[rot].transpose(1, 0, 2)
        ).astype(np.float16)
        x_rot = np.ascontiguousarray(common["x"][:, rot, :])
        efT = np.zeros((128, 3, E_SH), np.float16)
        eft_full = a * edge_feats[sl].T       # (300, 512), pre-scaled
        for i, (cch, o) in enumerate(zip(DCH, DOF)):
            efT[:cch, i, :] = eft_full[o:o + cch, :]
        in_maps.append(dict(inc=inc_pack, efT=efT,
                            **{k: v for k, v in common.items() if k != "x"},
                            x=x_rot))
    return in_maps


def kernel(node_feats, edge_feats, inc_mat, W_att, W_proj, alpha,
           ec_W_att, ec_W_proj, ec_b_proj, fc_W, fc_b, trace=False):
    nc = get_nc()
    in_maps = make_in_maps(node_feats, edge_feats, inc_mat, W_att, W_proj,
                           alpha, ec_W_att, ec_W_proj, ec_b_proj, fc_W, fc_b)
    res = run_bass_kernel_spmd(nc, in_maps, list(range(NCORES)), trace=trace)
    kernel.last_results = res
    parts = np.stack([np.asarray(r["out"], np.float64).reshape(8)
                      for r in res.results]).sum(axis=0)
    a = float(np.asarray(alpha))
    ec_W_proj = np.asarray(ec_W_proj, np.float64)
    b2 = np.asarray(ec_b_proj, np.float64) @ np.asarray(fc_W, np.float64) \
        + np.asarray(fc_b, np.float64)
    b2 = b2.reshape(NCAT)
    logits = parts[0:NCAT] / parts[NCAT] + b2
    return logits.astype(np.float32)


# revision 26
# speedup vs baseline: 1.0781x; 1.0282x over previous
"""HGConv fused kernel for one TRN2 chip (8 NeuronCores), SPMD via Bass/Tile.

Hardcoded for M=16384 nodes, E=4096 hyperedges, D=300, N_CAT=3, 8 cores.

Edge-sharded design (no mid-kernel ReduceScatter):
  - Core c owns edges [512c, 512(c+1)).  It streams the FULL node_feats X
    (fp16, replicated) plus its inc column-slice (fp16, host-pre-tiled so
    every DMA descriptor is a multi-KB contiguous run) and accumulates
    IX[e_c, :] = inc[:, e_c].T @ X locally in 4 PSUM banks (4 e-subtiles
    x 128 edges, contraction over all 128 m-tiles).
  - Tail on the 512 local edges: att = IX @ W_att (reassociated), softmax
    over d (stabilized), ef = IX * attn; ef2T = (1-a)*W_proj.T @ efT +
    a*edge_feats.T (edge_feats pre-transposed/pre-scaled on host);
    scores|G = ef2T.T @ [ec_W_att | ec_W_proj @ fc_W] (classifier weights
    folded on host); unstabilized exp(score) (scores are O(5), fp32-safe);
    p2|z accumulated with a PE matmul against [G | ones].
  - Each core writes its 8-float partial [p2(3) | z] as its output; the
    host gathers the 8 partials and finishes logits = sum(p2)/sum(z) + b2
    (b2 = ec_b @ fc_W + fc_b, folded).  No on-device collectives at all,
    so no core ever waits on a peer.
"""

import sys

for _p in ("/opt/trn_rl_repo", "/opt/pypackages"):
    if _p not in sys.path:
        sys.path.append(_p)

import numpy as np

import concourse.bacc as bacc
import concourse.tile as tile
from concourse import masks, mybir
from concourse.bass_utils import run_bass_kernel_spmd

F32 = mybir.dt.float32
F32R = mybir.dt.float32r
F16 = mybir.dt.float16
AX = mybir.AxisListType
OP = mybir.AluOpType
AF = mybir.ActivationFunctionType

NCORES = 8
M, E, D, NCAT = 16384, 4096, 300, 3
E_SH = E // NCORES          # 512 edges per core
ET_SH = E_SH // 128         # 4 e-subtiles per core
MT_TOT = M // 128           # 128 m-tiles over the full node axis
MCH = 8                     # m-tiles per streamed chunk
NCH = MT_TOT // MCH         # 16 chunks
DCH = (128, 128, 44)        # d split into partition chunks
DOF = (0, 128, 256)


def _build():
    nc = bacc.Bacc("TRN2", target_bir_lowering=False, debug=False,
                   num_devices=NCORES)
    x_d = nc.dram_tensor("x", [128, MT_TOT, D], F16, kind="ExternalInput")
    inc_d = nc.dram_tensor("inc", [128, MT_TOT, E_SH], F16,
                           kind="ExternalInput")
    efT_d = nc.dram_tensor("efT", [128, 3, E_SH], F16, kind="ExternalInput")
    watt_d = nc.dram_tensor("watt", [D, D], F16, kind="ExternalInput")
    wproj_d = nc.dram_tensor("wproj", [D, D], F16, kind="ExternalInput")
    sgw_d = nc.dram_tensor("sgw", [D, 4], F16, kind="ExternalInput")
    out_d = nc.dram_tensor("out", [1, 8], F32, kind="ExternalOutput")

    def mm(out, lhsT, rhs, start, stop):
        nc.tensor.matmul(out, lhsT, rhs, start=start, stop=stop)

    with tile.TileContext(nc) as tc, \
         tc.tile_pool(name="sb", bufs=1) as sb, \
         tc.tile_pool(name="dram", bufs=1, space="DRAM") as dram:

        # ---------- phase 1: IX[e_c, :] = inc_c.T @ X (full m) ----------
        with tc.tile_pool(name="pacc", bufs=ET_SH, space="PSUM") as pacc, \
             tc.tile_pool(name="xp", bufs=8) as xp, \
             tc.tile_pool(name="ip", bufs=8) as ip:
            acc = [pacc.tile([128, D], F32, name=f"acc{es}", tag="acc")
                   for es in range(ET_SH)]
            # small leading chunks so the PE starts as soon as possible
            sizes = [2, 2, 4] + [MCH] * ((MT_TOT - 8) // MCH)
            assert sum(sizes) == MT_TOT
            mt0 = 0
            for ch, csz in enumerate(sizes):
                i_t = ip.tile([128, csz, E_SH], F16, tag="inc",
                              name=f"inc_c{ch}")
                x_t = xp.tile([128, csz, D], F16, tag="x", name=f"x_c{ch}")
                nc.sync.dma_start(i_t[:], inc_d[:, mt0:mt0 + csz, :])
                nc.scalar.dma_start(x_t[:], x_d[:, mt0:mt0 + csz, :])
                for mt in range(csz):
                    for es in range(ET_SH):
                        mm(acc[es][:],
                           i_t[:, mt, es * 128:(es + 1) * 128],
                           x_t[:, mt, :],
                           start=(mt0 + mt == 0),
                           stop=(mt0 + mt == MT_TOT - 1))
                mt0 += csz

            # ---------- small weights / constants (overlap phase 1) ------
            watt_sb = sb.tile([128, 3, D], F16)
            wproj_sb = sb.tile([128, 3, D], F16)
            sgw_sb = sb.tile([128, 3, 4], F16)
            for i, (c, o) in enumerate(zip(DCH, DOF)):
                nc.gpsimd.dma_start(watt_sb[:c, i, :], watt_d[o:o + c, :])
                nc.gpsimd.dma_start(wproj_sb[:c, i, :], wproj_d[o:o + c, :])
                nc.gpsimd.dma_start(sgw_sb[:c, i, :], sgw_d[o:o + c, :])
            efT_sb = sb.tile([128, 3, E_SH], F16)
            nc.gpsimd.dma_start(efT_sb[:], efT_d[:])
            ident = sb.tile([128, 128], F16)
            masks.make_identity(nc, ident[:])

            # IX psum -> sbuf (inside pacc scope, then release its banks)
            def cp(k, dst, src):
                # psum-reading copies: only ACT/DVE may touch PSUM
                e = (nc.scalar.copy, nc.vector.tensor_copy)[k % 2]
                e(dst, src)

            DP = 384                        # d padded to a 128 multiple
            ix_sb = sb.tile([128, ET_SH, DP], F16)
            nc.vector.memset(ix_sb[:, :, D:DP], 0.0)
            for es in range(ET_SH):
                cp(es, ix_sb[:, es, 0:D], acc[es][:])

        # ---------- tail on this core's 512 edges ----------
        if True:
            with tc.tile_pool(name="pp", bufs=4, space="PSUM") as pp:
                def transpose_512xD(src_sb, dstT_sb):
                    # src (128, 4, 384) f16 [e-part] -> dstT (128, 3, 512)
                    # f16 [d-part]; PE transpose (f16, 1cyc/row) + cast copy
                    k = 0
                    for et in range(ET_SH):
                        for i, (c, o) in enumerate(zip(DCH, DOF)):
                            tp = pp.tile([128, 128], F16, tag="ps")
                            nc.tensor.transpose(tp[:c, :128],
                                                src_sb[:, et, o:o + c],
                                                ident[:])
                            cp(k, dstT_sb[:c, i, et * 128:(et + 1) * 128],
                               tp[:c, :128])
                            k += 1

                ixT_sb = sb.tile([128, 3, E_SH], F16)
                transpose_512xD(ix_sb, ixT_sb)

                # edge_att = IX @ W_att; softmax over d; ef = IX * attn
                ef_sb = sb.tile([128, ET_SH, DP], F16)
                nc.vector.memset(ef_sb[:, :, D:DP], 0.0)
                stat_sb = sb.tile([128, ET_SH, 4], F32)
                for et in range(ET_SH):
                    att = pp.tile([128, D], F32, tag="ps")
                    for i, c in enumerate(DCH):
                        mm(att[:], ixT_sb[:c, i, et * 128:(et + 1) * 128],
                           watt_sb[:c, i, :], start=(i == 0), stop=(i == 2))
                    nmax = stat_sb[:, et, 0:1]
                    nc.vector.tensor_reduce(nmax, att[:], axis=AX.X,
                                            op=OP.max, negate=True)
                    ex = pp.tile([128, D], F32, tag="ps")
                    rsum = stat_sb[:, et, 1:2]
                    nc.scalar.activation(ex[:], att[:], AF.Exp, bias=nmax,
                                         scale=1.0, accum_out=rsum)
                    rcp = stat_sb[:, et, 2:3]
                    nc.vector.reciprocal(rcp, rsum)
                    nc.vector.scalar_tensor_tensor(
                        ef_sb[:, et, 0:D], ex[:], rcp, ix_sb[:, et, 0:D],
                        op0=OP.mult, op1=OP.mult)

                efTT_sb = sb.tile([128, 3, E_SH], F16)
                transpose_512xD(ef_sb, efTT_sb)

                # ef2T = (1-a)*W_proj.T @ efT + a*edge_feats.T
                ef2T_sb = sb.tile([128, 3, E_SH], F16)
                for i, (c, o) in enumerate(zip(DCH, DOF)):
                    pj = pp.tile([128, E_SH], F32, tag="ps")
                    for j, cj in enumerate(DCH):
                        mm(pj[:c, :], wproj_sb[:cj, j, o:o + c],
                           efTT_sb[:cj, j, :], start=(j == 0),
                           stop=(j == 2))
                    nc.vector.tensor_add(ef2T_sb[:c, i, :], pj[:c, :],
                                         efT_sb[:c, i, :])

                # scores|G = ef2 @ [ec_W_att | ec_W_proj @ fc_W]  -> (e, 4)
                g_sb = sb.tile([128, ET_SH, 8], F32)
                nc.vector.memset(g_sb[:, :, 4:5], 1.0)
                expw_sb = sb.tile([128, ET_SH], F32)
                for et in range(ET_SH):
                    sg = pp.tile([128, 4], F32, tag="ps")
                    for j, cj in enumerate(DCH):
                        mm(sg[:], ef2T_sb[:cj, j, et * 128:(et + 1) * 128],
                           sgw_sb[:cj, j, :], start=(j == 0), stop=(j == 2))
                    nc.scalar.copy(g_sb[:, et, 0:4], sg[:])
                    nc.scalar.activation(expw_sb[:, et:et + 1],
                                         g_sb[:, et, 0:1], AF.Exp, scale=1.0)

                # p2|z = sum_e exp_e * [G_e | 1]   (PE contraction over e)
                p2 = pp.tile([1, 4], F32, tag="p2")
                for et in range(ET_SH):
                    mm(p2[:], expw_sb[:, et:et + 1], g_sb[:, et, 1:5],
                       start=(et == 0), stop=(et == ET_SH - 1))
                # per-core partial [p2(3) | z | pad]; host sums over cores
                # and finishes logits = sum(p2)/sum(z) + b2 (gather/unshard)
                pk_sb = sb.tile([1, 8], F32)
                nc.vector.memset(pk_sb[:, 4:8], 0.0)
                nc.scalar.copy(pk_sb[:, 0:4], p2[:])
                nc.sync.dma_start(out_d[:], pk_sb[0:1, :])

    nc.compile()
    return nc


_CACHE = {}


def get_nc():
    if "nc" not in _CACHE:
        _CACHE["nc"] = _build()
    return _CACHE["nc"]


def make_in_maps(node_feats, edge_feats, inc_mat, W_att, W_proj, alpha,
                 ec_W_att, ec_W_proj, ec_b_proj, fc_W, fc_b):
    cc = lambda a: np.ascontiguousarray(np.asarray(a, np.float32))
    node_feats = cc(node_feats)
    inc_mat = cc(inc_mat)
    edge_feats = cc(edge_feats)
    W_att, W_proj = cc(W_att), cc(W_proj)
    ec_W_att, ec_W_proj = cc(ec_W_att).reshape(D, 1), cc(ec_W_proj)
    ec_b_proj, fc_W, fc_b = cc(ec_b_proj), cc(fc_W), cc(fc_b)
    a = float(np.asarray(alpha))

    # x packed [p, mt, d] fp16, replicated
    x_pack = np.ascontiguousarray(
        node_feats.reshape(MT_TOT, 128, D).transpose(1, 0, 2)
    ).astype(np.float16)
    # folded weights
    G2 = ec_W_proj @ fc_W                     # (300, 3)
    sgw = np.ascontiguousarray(
        np.concatenate([ec_W_att, G2], axis=1))  # (300, 4)
    wproj_s = np.ascontiguousarray((1.0 - a) * W_proj).astype(np.float16)
    common = dict(x=x_pack, watt=W_att.astype(np.float16), wproj=wproj_s,
                  sgw=sgw.astype(np.float16))

    in_maps = []
    for c in range(NCORES):
        sl = slice(c * E_SH, (c + 1) * E_SH)
        # rotate the m-tile order per core so the 8 cores never stream the
        # same region of the replicated x at the same instant (HBM hotspot)
        rot = np.roll(np.arange(MT_TOT), -c * (MT_TOT // NCORES))
        inc_pack = np.ascontiguousarray(
            inc_mat[:, sl].reshape(MT_TOT, 128, E_SH)[rot].transpose(1, 0, 2)
        ).astype(np.float16)
        x_rot = np.ascontiguousarray(common["x"][:, rot, :])
        efT = np.zeros((128, 3, E_SH), np.float16)
        eft_full = a * edge_feats[sl].T       # (300, 512), pre-scaled
        for i, (cch, o) in enumerate(zip(DCH, DOF)):
            efT[:cch, i, :] = eft_full[o:o + cch, :]
        in_maps.append(dict(inc=inc_pack, efT=efT,
                            **{k: v for k, v in common.items() if k != "x"},
                            x=x_rot))
    return in_maps


def kernel(node_feats, edge_feats, inc_mat, W_att, W_proj, alpha,
           ec_W_att, ec_W_proj, ec_b_proj, fc_W, fc_b, trace=False):
    nc = get_nc()
    in_maps = make_in_maps(node_feats, edge_feats, inc_mat, W_att, W_proj,
                           alpha, ec_W_att, ec_W_proj, ec_b_proj, fc_W, fc_b)
    res = run_bass_kernel_spmd(nc, in_maps, list(range(NCORES)), trace=trace)
    kernel.last_results = res
    parts = np.stack([np.asarray(r["out"], np.float64).reshape(8)
                      for r in res.results]).sum(axis=0)
    b2 = (np.asarray(ec_b_proj, np.float64) @ np.asarray(fc_W, np.float64)
          + np.asarray(fc_b, np.float64)).reshape(NCAT)
    logits = parts[0:NCAT] / parts[NCAT] + b2
    return logits.astype(np.float32)
